# revision 1
# baseline (speedup 1.0000x reference)
"""Biquad lowpass IIR filter (torchaudio lowpass_biquad) on 8 Trainium2 cores.

Full input: clip [128, 160000] f32. Output same shape.

Math: with SR=32000, cutoff=8000, Q=0.707 -> w0 = pi/2, cos(w0) ~ 0, so
  a1 ~ 1e-17 (negligible), b1 = 2*b0, b2 = b0 (exactly, in f32).
The filter reduces to
  y[n] = t[n] - a2*y[n-2],   t[n] = b0*(x[n] + 2x[n-1] + x[n-2])
which splits into two independent lag-1 linear scans over the even/odd
subsequences - natively supported by the DVE tensor_tensor_scan
(state = data0*state + data1).

Sharding: data-parallel over batch, 16 clips/core. Each clip is further
split into 8 segments of 20000 so a core fills 128 partitions. Segment /
chunk boundary state is handled with a W-sample warm-up region re-read
from DRAM: the recurrence forgets its initial condition at rate
a2^(W/2) ~ 6e-13 (a2 ~ 0.1715), far below f32 noise.
"""

import math

import numpy as np

import concourse.bacc as bacc
import concourse.mybir as mybir
import concourse.tile as tile
from concourse import bass_utils

F32 = mybir.dt.float32

B = 128          # batch (full)
T = 160000       # samples per clip
N_CORES = 8
CPC = B // N_CORES   # clips per core = 16
SEGS = 8             # segments per clip -> CPC*SEGS = 128 partitions
S = T // SEGS        # segment length = 20000
F = 2000             # chunk columns
NCHUNK = S // F      # 5
W = 34               # left context: 32 warm-up + 2 FIR taps
E = F + W            # extended chunk width

SAMPLE_RATE = 32000.0
CUTOFF = 8000.0
Q = 0.707


def _coeffs():
    # identical arithmetic to the reference implementation
    w0 = 2.0 * math.pi * CUTOFF / SAMPLE_RATE
    alpha = math.sin(w0) / (2.0 * Q)
    cos_w0 = math.cos(w0)
    b0 = (1.0 - cos_w0) / 2.0
    a0 = 1.0 + alpha
    a2 = 1.0 - alpha
    return float(np.float32(b0 / a0)), float(np.float32(a2 / a0))


def build_bass():
    b0n, a2n = _coeffs()
    nc = bacc.Bacc(
        "TRN2",
        target_bir_lowering=False,
        debug=False,
        enable_asserts=False,
        num_devices=N_CORES,
    )
    x = nc.dram_tensor("x", [CPC, T], F32, kind="ExternalInput").ap()
    y = nc.dram_tensor("y", [CPC, T], F32, kind="ExternalOutput").ap()

    # partition p = seg*CPC + clip
    xr = x.rearrange("c (s t) -> s c t", s=SEGS)
    yr = y.rearrange("c (s t) -> s c t", s=SEGS)

    half = (E - 2) // 2

    with tile.TileContext(nc) as tc:
        with (
            tc.tile_pool(name="xtp", bufs=3) as xtp,
            tc.tile_pool(name="tmp", bufs=3) as tmp,
            tc.tile_pool(name="up", bufs=3) as up,
            tc.tile_pool(name="const", bufs=1) as cpool,
        ):
            cneg_a2 = cpool.tile([128, half], F32)
            nc.vector.memset(cneg_a2[:, :], -a2n)

            for k in range(NCHUNK):
                xt = xtp.tile([128, E], F32, tag="xt")
                if k == 0:
                    # first 34 cols: zeros for seg 0 (true zero initial
                    # conditions), previous-segment tail for segs 1..7
                    nc.vector.memset(xt[0:CPC, 0:W], 0.0)
                    nc.sync.dma_start(xt[CPC:128, 0:W], xr[0 : SEGS - 1, :, S - W : S])
                    nc.sync.dma_start(xt[:, W:E], xr[:, :, 0:F])
                else:
                    base = k * F - W
                    nc.sync.dma_start(xt[:, :], xr[:, :, base : base + E])

                # xb = b0 * x  (ACT -- its ports are independent of DVE's)
                xb = tmp.tile([128, E], F32, tag="xb")
                if k == 0:
                    nc.scalar.mul(xb[:, 0:W], xt[:, 0:W], b0n)
                    nc.scalar.mul(xb[:, W:E], xt[:, W:E], b0n)
                else:
                    nc.scalar.mul(xb[:, :], xt[:, :], b0n)

                # s[j] = xb[j+1] + xb[j]  (DVE; gpsimd shares DVE's SBUF
                # ports so offloading there adds no throughput)
                s = tmp.tile([128, E - 1], F32, tag="s")
                nc.vector.tensor_add(s[:, :], xb[:, 1:E], xb[:, 0 : E - 1])

                # t[j] = s[j+1] + s[j]  (DVE)
                t = tmp.tile([128, E - 2], F32, tag="t")
                nc.vector.tensor_add(t[:, :], s[:, 1 : E - 1], s[:, 0 : E - 2])

                # u = scan(t): state = -a2*state + t, over even/odd cols
                u = up.tile([128, E - 2], F32, tag="u")
                nc.vector.tensor_tensor_scan(
                    u[:, 0 : E - 2 : 2],
                    cneg_a2[:, 0:half],
                    t[:, 0 : E - 2 : 2],
                    0.0,
                    mybir.AluOpType.mult,
                    mybir.AluOpType.add,
                )
                nc.vector.tensor_tensor_scan(
                    u[:, 1 : E - 2 : 2],
                    cneg_a2[:, 0:half],
                    t[:, 1 : E - 2 : 2],
                    0.0,
                    mybir.AluOpType.mult,
                    mybir.AluOpType.add,
                )

                nc.sync.dma_start(
                    yr[:, :, k * F : (k + 1) * F], u[:, W - 2 : W - 2 + F]
                )
    nc.compile()
    return nc


_cached = {}


def _run(clip: np.ndarray, trace: bool = False):
    clip = np.ascontiguousarray(np.asarray(clip, dtype=np.float32))
    assert clip.shape == (B, T)
    if "nc" not in _cached:
        _cached["nc"] = build_bass()
    nc = _cached["nc"]
    in_maps = [
        {"x": np.ascontiguousarray(clip[i * CPC : (i + 1) * CPC])}
        for i in range(N_CORES)
    ]
    res = bass_utils.run_bass_kernel_spmd(
        nc, in_maps, list(range(N_CORES)), trace=trace
    )
    out = np.concatenate([res.results[i]["y"] for i in range(N_CORES)], axis=0)
    return out, res


def kernel(clip: np.ndarray) -> np.ndarray:
    out, _ = _run(clip, trace=False)
    return out

